# revision 6
# baseline (speedup 1.0000x reference)
"""Trainium2 Bass kernel for Gaussian KDE evaluation.

reference math:
    val[m] = (1/N) * sum_n exp(t1 - 0.5*d2(m,n)/bw^2)
    d2(m,n) = |e_m|^2 + |b_n|^2 - 2<e_m, b_n>
    t1 = -0.5*D*log(2*pi) - log_bw,  bw^2 = exp(2*log_bw)

Strategy (8 NeuronCores, x_eval row-sharded, x_base/log_bw replicated):
  Per core, one K=66 bf16 matmul per output tile produces |b|^2 - 2<e,b>
  in PSUM at 1 PE-cycle/column.  Full fp32-like accuracy comes from a
  hi/lo bf16 split: with E = -2*x_eval, the 66 stationary rows are
  [E_hi; E_lo; E_lo; E_hi; 1; 1] against moving rows
  [b_hi; b_lo; b_hi; b_lo; |b|^2_hi; |b|^2_lo], so the contraction sums
  all four cross products (E_hi+E_lo)(b_hi+b_lo) plus the |b|^2 row.
  A single ScalarE ACTIVATE then computes exp(scale*psum + bias) in
  place (scale = -0.5/bw^2, per-partition bias = t1 - ln(N) +
  scale*|e_m|^2) and its accum_out emits the row-sum.

  x_base is transposed once up front into a persistent [66, N] bf16
  SBUF buffer so the steady-state loop is pure matmul + activation;
  PSUM is split into two 4-bank [128, 2048] buffers so the Exp
  activation on chunk k overlaps the matmuls of chunk k+1.
"""

import numpy as np

M, N, D = 8192, 16384, 16
NCORES = 8
MS = M // NCORES          # eval rows per core
RT = MS // 128            # row tiles per core (128 evals each)
CH = 2048                 # column chunk: 4 PSUM banks
NCH = N // CH
KK = 66                   # contraction rows: 4x16 hi/lo blocks + 2 sqb rows
LOG_2PI = float(np.log(2.0 * np.pi))

_CACHE = {}


def _build_nc():
    from concourse import bacc, mybir, masks, tile

    f32 = mybir.dt.float32
    bf16 = mybir.dt.bfloat16
    nc = bacc.Bacc("TRN2", target_bir_lowering=False, debug=False,
                   num_devices=NCORES)

    x_eval = nc.dram_tensor("x_eval", [MS, D], f32, kind="ExternalInput")
    x_base = nc.dram_tensor("x_base", [N, D], f32, kind="ExternalInput")
    log_bw = nc.dram_tensor("log_bw", [1, 1], f32, kind="ExternalInput")
    out = nc.dram_tensor("out", [128, RT], f32, kind="ExternalOutput")
    sqb_hi_d = nc.dram_tensor("sqb_hi_d", [1, N], bf16)
    sqb_lo_d = nc.dram_tensor("sqb_lo_d", [1, N], bf16)

    NBT = N // 128            # number of 128-row base tiles
    Exp = mybir.ActivationFunctionType.Exp
    ADD = mybir.AluOpType.add
    MULT = mybir.AluOpType.mult
    SUB = mybir.AluOpType.subtract
    X = mybir.AxisListType.X
    # constant part of the bias: t1 - ln(N); -log_bw and the |e|^2 term are
    # added on-device.
    c0 = -0.5 * D * LOG_2PI - float(np.log(N))

    with tile.TileContext(nc) as tc:
        with (
            tc.tile_pool(name="persist", bufs=1) as pp,
            tc.tile_pool(name="mm", bufs=2, space="PSUM") as mmp,
        ):
            identity = pp.tile([128, 128], f32)
            masks.make_identity(nc, identity[:])
            identity_b = pp.tile([128, 128], bf16)
            masks.make_identity(nc, identity_b[:])

            # ---- log_bw -> per-partition scale/bias columns -------------
            ones_row = pp.tile([1, 128], f32)
            nc.vector.memset(ones_row[:], 1.0)
            lb_sb = pp.tile([1, 1], f32)
            nc.sync.dma_start(out=lb_sb[:], in_=log_bw[:])
            ps_lb = mmp.tile([128, CH], f32, tag="mm")
            nc.tensor.matmul(ps_lb[:, 0:1], ones_row[:], lb_sb[:],
                             start=True, stop=True)
            # scale = -0.5 * exp(-2*log_bw)
            inv_bw2 = pp.tile([128, 1], f32)
            nc.scalar.activation(inv_bw2[:], ps_lb[:, 0:1], Exp, scale=-2.0)
            scale_col = pp.tile([128, 1], f32)
            nc.vector.tensor_scalar_mul(scale_col[:], inv_bw2[:], -0.5)
            # c_col = c0 - log_bw
            c_col = pp.tile([128, 1], f32)
            nc.vector.tensor_scalar(out=c_col[:], in0=ps_lb[:, 0:1],
                                    scalar1=-1.0, scalar2=c0,
                                    op0=MULT, op1=ADD)

            # ---- eval-side setup ----------------------------------------
            ev_nat = pp.tile([128, RT * D], f32)
            nc.sync.dma_start(
                out=ev_nat[:].rearrange("p (t d) -> p t d", d=D),
                in_=x_eval[:].rearrange("(p t) d -> p t d", p=128))
            ev_sq = pp.tile([128, RT * D], f32)
            nc.vector.tensor_mul(ev_sq[:], ev_nat[:], ev_nat[:])
            sq_e = pp.tile([128, RT], f32)
            nc.vector.tensor_reduce(
                out=sq_e[:], in_=ev_sq[:].rearrange("p (t d) -> p t d", d=D),
                axis=X, op=ADD)
            # bias_all[:, rt] = scale*|e|^2 + (c0 - log_bw)
            bias_all = pp.tile([128, RT], f32)
            nc.vector.tensor_scalar(out=bias_all[:], in0=sq_e[:],
                                    scalar1=scale_col[:, 0:1],
                                    scalar2=c_col[:, 0:1],
                                    op0=MULT, op1=ADD)

            # evT: 66 rows [E_hi; E_lo; E_lo; E_hi; 1; 1], E = -2*eval.
            # Built by transposing a [128, 66]-per-rowtile bf16 block so no
            # engine writes at a non-32-aligned partition offset.
            E_nat = pp.tile([128, RT * D], f32)
            nc.vector.tensor_scalar_mul(E_nat[:], ev_nat[:], -2.0)
            ev_aug = pp.tile([128, RT * KK], bf16)
            nc.vector.memset(ev_aug[:], 1.0)
            ea = ev_aug[:].rearrange("p (t c) -> p t c", c=KK)
            en = E_nat[:].rearrange("p (t d) -> p t d", d=D)
            nc.vector.tensor_copy(ea[:, :, 0:16], en)                # E_hi
            nc.vector.scalar_tensor_tensor(                          # E_lo
                out=ea[:, :, 16:32], in0=en, scalar=1.0,
                in1=ea[:, :, 0:16], op0=MULT, op1=SUB)
            nc.vector.tensor_copy(ea[:, :, 32:48], ea[:, :, 16:32])  # E_lo
            nc.vector.tensor_copy(ea[:, :, 48:64], ea[:, :, 0:16])   # E_hi
            evT = pp.tile([KK, MS], bf16)
            for rt in range(RT):
                ps_e = mmp.tile([128, CH], bf16, tag="mm")
                nc.tensor.transpose(ps_e[0:KK, 0:128],
                                    ev_aug[:, rt * KK:(rt + 1) * KK],
                                    identity_b[:])
                nc.vector.tensor_copy(
                    evT[0:KK, rt * 128:(rt + 1) * 128], ps_e[0:KK, 0:128])

            # ---- base load, |b|^2, and hi/lo split ----------------------
            bs_nat = pp.tile([128, NBT * D], f32)
            nc.sync.dma_start(
                out=bs_nat[:].rearrange("p (t d) -> p t d", d=D),
                in_=x_base[:].rearrange("(p t) d -> p t d", p=128))
            bs_sq = pp.tile([128, NBT * D], f32)
            nc.vector.tensor_mul(bs_sq[:], bs_nat[:], bs_nat[:])
            sq_b = pp.tile([128, NBT], f32)
            nc.vector.tensor_reduce(
                out=sq_b[:], in_=bs_sq[:].rearrange("p (t d) -> p t d", d=D),
                axis=X, op=ADD)
            # |b|^2 row: transpose, split hi/lo, bounce through DRAM to get
            # a [1, N] row layout.
            ps_sqb = mmp.tile([128, CH], f32, tag="mm")
            nc.tensor.transpose(ps_sqb[:, 0:128], sq_b[:], identity[:])
            sqbT_hi = pp.tile([128, 128], bf16)
            nc.vector.tensor_copy(sqbT_hi[:], ps_sqb[:, 0:128])
            sqbT_lo = pp.tile([128, 128], bf16)
            nc.vector.scalar_tensor_tensor(
                out=sqbT_lo[:], in0=ps_sqb[:, 0:128], scalar=1.0,
                in1=sqbT_hi[:], op0=MULT, op1=SUB)
            nc.sync.dma_start(
                out=sqb_hi_d[:].rearrange("o (t p) -> (o t) p", p=128),
                in_=sqbT_hi[:])
            nc.sync.dma_start(
                out=sqb_lo_d[:].rearrange("o (t p) -> (o t) p", p=128),
                in_=sqbT_lo[:])

            # bs_aug: per base tile, 32 bf16 columns [b_hi | b_lo]
            bs_aug = pp.tile([128, NBT * 32], bf16)
            ba = bs_aug[:].rearrange("p (t c) -> p t c", c=32)
            bn = bs_nat[:].rearrange("p (t d) -> p t d", d=D)
            nc.vector.tensor_copy(ba[:, :, 0:16], bn)                # b_hi
            nc.vector.scalar_tensor_tensor(                          # b_lo
                out=ba[:, :, 16:32], in0=bn, scalar=1.0,
                in1=ba[:, :, 0:16], op0=MULT, op1=SUB)

            # persistent transposed base: rows [b_hi; b_lo] twice + sqb rows
            rhsT = pp.tile([KK, N], bf16)
            nc.sync.dma_start(out=rhsT[64:65, 0:N], in_=sqb_hi_d[0:1, 0:N])
            nc.sync.dma_start(out=rhsT[65:66, 0:N], in_=sqb_lo_d[0:1, 0:N])
            for g in range(NBT // 16):          # 16 transposes per PSUM buf
                ps_t = mmp.tile([128, CH], bf16, tag="mm")
                for k in range(16):
                    t = g * 16 + k
                    nc.tensor.transpose(
                        ps_t[0:32, k * 128:(k + 1) * 128],
                        bs_aug[:, t * 32:(t + 1) * 32], identity_b[:])
                nc.vector.tensor_copy(
                    rhsT[0:32, g * CH:(g + 1) * CH], ps_t[0:32, 0:CH])
                nc.vector.tensor_copy(
                    rhsT[32:64, g * CH:(g + 1) * CH], ps_t[0:32, 0:CH])

            # ---- main loop ----------------------------------------------
            sums = pp.tile([128, RT * NCH], f32)
            for ci in range(NCH):
                cs = ci * CH
                for rt in range(RT):
                    ps = mmp.tile([128, CH], f32, tag="mm")
                    for j in range(CH // 512):
                        nc.tensor.matmul(
                            ps[:, j * 512:(j + 1) * 512],
                            evT[0:KK, rt * 128:(rt + 1) * 128],
                            rhsT[0:KK, cs + j * 512:cs + (j + 1) * 512],
                            start=True, stop=True)
                    nc.scalar.activation(
                        ps[:, 0:CH], ps[:, 0:CH], Exp,
                        bias=bias_all[:, rt:rt + 1],
                        scale=scale_col[:, 0:1],
                        accum_out=sums[:, rt * NCH + ci:rt * NCH + ci + 1])

            # ---- finalize -----------------------------------------------
            val = pp.tile([128, RT], f32)
            for rt in range(RT):
                nc.vector.tensor_reduce(
                    out=val[:, rt:rt + 1],
                    in_=sums[:, rt * NCH:(rt + 1) * NCH], axis=X, op=ADD)
            nc.sync.dma_start(out=out[:], in_=val[:])

    nc.compile()
    return nc


def kernel(x_eval, x_base, log_bw):
    from concourse.bass_utils import run_bass_kernel_spmd

    if "nc" not in _CACHE:
        _CACHE["nc"] = _build_nc()
    nc = _CACHE["nc"]

    x_eval = np.ascontiguousarray(x_eval, dtype=np.float32)
    x_base = np.ascontiguousarray(x_base, dtype=np.float32)
    lb = np.asarray(log_bw, dtype=np.float32).reshape(1, 1)
    in_maps = [
        {
            "x_eval": x_eval[i * MS:(i + 1) * MS],
            "x_base": x_base,
            "log_bw": lb,
        }
        for i in range(NCORES)
    ]
    res = run_bass_kernel_spmd(nc, in_maps, list(range(NCORES)))
    # out[p, rt] holds eval point p*RT + rt of the shard -> row-major flatten
    shards = [r["out"].reshape(-1) for r in res.results]
    return np.concatenate(shards).astype(np.float32)
